# revision 24
# baseline (speedup 1.0000x reference)
"""Causal multi-head self-attention on 8 Trainium2 NeuronCores.

Problem: x[4, 2048, 1024], w_qkv[3072, 1024], w_proj[1024, 1024], b_proj[1024].
y = softmax_causal((xWq)(xWk)^T / 8) (xWv) -> concat heads -> @ w_proj^T + b.

Sharding: 8 cores = (batch b in 0..3) x (head-group g in 0..1), 8 heads per
core.  Each core computes the full attention for its (b, g) and a partial
output projection over its 512 head-dims; the host sums the two per-batch
partials, adds bias, and transposes back.

Everything on-chip lives in "transposed" space (feature dim on partitions):
the host passes x^T and pre-transposed weight slices, so the kernel needs no
on-chip transposes.  All matmuls use float32r (TF32-rate, ~1e-4 rel err).

Per-core dataflow, per t-chunk j of 512 tokens (j = 0..3):
  qkv:   qT/kT tiles  [64d x 2 heads, 512t] = W_qk^T-chunk.T @ xT-chunk
         V_aug tiles  [128t, 8 heads, 65]   = (xT-tile).T @ Wv^T  (+ ones col)
  attn (q-chunk = j, per head pair, per k-tile of 128):
         S^T [128k, Nq] = kT-slice.T @ qT-slice     (K=64, bp 0/64)
         E^T = exp(S^T/8)  (ScalarE, PSUM->SBUF, f32r out)
         tri-mask multiply on diagonal 128x128 sub-block (VectorE)
         O_aug [65, Nq] += V_aug.T @ E^T   (row 64 = softmax denominator)
  norm:  r = O_aug[64]; 1/r via DVE; broadcast over partitions via GpSimd;
         O_norm^T = O_aug[0:64] * (1/r)   -> staged [128, 512] per head pair
  proj:  yT [128o, 512q] += Wp^T-chunk.T @ O_norm^T-pair  -> DMA to DRAM
"""

import numpy as np

B, T, C = 4, 2048, 1024
HG, D = 8, 64           # heads per core, head dim
NCH = T // 512          # 4 t-chunks of 512
NKT = T // 128          # 16 k-tiles of 128
CCH = C // 128          # 8 contraction chunks for the qkv projection

_CACHE = {}


def _build():
    import concourse.bacc as bacc
    import concourse.mybir as mybir
    import concourse.tile as tile
    from contextlib import ExitStack

    F32 = mybir.dt.float32
    F32R = mybir.dt.float32r
    EXP = mybir.ActivationFunctionType.Exp

    nc = bacc.Bacc("TRN2", target_bir_lowering=False, debug=False)

    xT = nc.dram_tensor("xT", [C, T], F32R, kind="ExternalInput").ap()
    wqkT = nc.dram_tensor("wqkT", [C, 1024], F32R, kind="ExternalInput").ap()
    wvT = nc.dram_tensor("wvT", [C, 512], F32R, kind="ExternalInput").ap()
    wpT = nc.dram_tensor("wpT", [512, 1024], F32R, kind="ExternalInput").ap()
    mask = nc.dram_tensor("mask", [128, 128], F32, kind="ExternalInput").ap()
    onesv = nc.dram_tensor("onesv", [128, HG], F32R, kind="ExternalInput").ap()
    yT = nc.dram_tensor("yT", [C, T], F32, kind="ExternalOutput").ap()

    with tile.TileContext(nc) as tc, ExitStack() as ctx:
        consts = ctx.enter_context(tc.tile_pool(name="consts", bufs=1))
        xt_pool = ctx.enter_context(tc.tile_pool(name="xt", bufs=1))
        qt_pool = ctx.enter_context(tc.tile_pool(name="qt", bufs=2))
        kv_pool = ctx.enter_context(tc.tile_pool(name="kv", bufs=1))
        e_pool = ctx.enter_context(tc.tile_pool(name="e", bufs=3))
        o_pool = ctx.enter_context(tc.tile_pool(name="o", bufs=8))
        r_pool = ctx.enter_context(tc.tile_pool(name="r", bufs=1))
        rb_pool = ctx.enter_context(tc.tile_pool(name="rb", bufs=1))
        y_sb = ctx.enter_context(tc.tile_pool(name="y_sb", bufs=2))

        # 8 psum banks: s-pair tiles 2x[128,1024] (4) + shared qkv/proj
        # accumulators (2) + O-pair accumulators (2)
        acc_ps = ctx.enter_context(tc.tile_pool(name="acc_ps", bufs=1, space="PSUM"))
        s_ps = ctx.enter_context(tc.tile_pool(name="s_ps", bufs=2, space="PSUM"))
        o_ps = ctx.enter_context(tc.tile_pool(name="o_ps", bufs=3, space="PSUM"))

        # ---- resident constants / weights ----
        # split weight loads per output-column group so the first qkv matmul
        # group only waits for its own 512KB slice
        wqk_sb = consts.tile([128, CCH, 1024], F32R)
        wqkT_r = wqkT.rearrange("(cc p) o -> p cc o", p=128)
        wv_sb = consts.tile([128, CCH, 512], F32R)
        wvT_r = wvT.rearrange("(cc p) o -> p cc o", p=128)
        # weights go on idle engines' DMA queues so the sync queue leads
        # with the x chunks the first matmuls need
        for ot in range(8):
            nc.gpsimd.dma_start(wqk_sb[:, :, ot * 128:(ot + 1) * 128],
                                wqkT_r[:, :, ot * 128:(ot + 1) * 128])
        for cc in range(CCH):
            nc.gpsimd.dma_start(wv_sb[:, cc, :], wvT_r[:, cc, :])
        wp_sb = consts.tile([128, 4, 1024], F32R)
        nc.gpsimd.dma_start(wp_sb[:], wpT.rearrange("(hh p) o -> p hh o", p=128))
        mask_sb = consts.tile([128, 128], F32)
        nc.gpsimd.dma_start(mask_sb[:], mask)

        # K (all chunks) and V_aug (all t-tiles) stay resident
        kT_sb = consts.tile([128, 4, T], F32R)            # [2 heads*64d, pair, t]
        vaug = consts.tile([128, NKT, HG, 65], F32R)      # [t, k-tile, head, d+1]

        xT_r = xT.rearrange("(cc p) t -> p cc t", p=128)

        def qkv_chunk_groups(j):
            """Returns (qt_tile, [group closures]) for t-chunk j.  Each
            closure emits one 8-matmul psum accumulation group; calling them
            interleaved with attention keeps the PE dense."""
            xt = xt_pool.tile([128, CCH, 512], F32R, name="xt", tag="xt")
            for cc in range(CCH):
                nc.sync.dma_start(xt[:, cc, :],
                                  xT_r[:, cc, j * 512:(j + 1) * 512])
            qt = qt_pool.tile([128, 4, 512], F32R, name="qt", tag="qt")

            def qk_group(ot):
                ps = acc_ps.tile([128, 512], F32, name="acc", tag="acc")
                for cc in range(CCH):
                    nc.tensor.matmul(
                        ps[:], wqk_sb[:, cc, ot * 128:(ot + 1) * 128],
                        xt[:, cc, :], start=(cc == 0), stop=(cc == CCH - 1),
                    )
                if ot < 4:
                    nc.vector.tensor_copy(qt[:, ot, :], ps[:])
                else:
                    nc.vector.tensor_copy(
                        kT_sb[:, ot - 4, j * 512:(j + 1) * 512], ps[:])

            def v_group(tt):
                tj = 4 * j + tt
                ps = acc_ps.tile([128, 512], F32, name="acc", tag="acc")
                for cc in range(CCH):
                    nc.tensor.matmul(
                        ps[:], xt[:, cc, tt * 128:(tt + 1) * 128],
                        wv_sb[:, cc, :], start=(cc == 0), stop=(cc == CCH - 1),
                    )
                nc.vector.tensor_copy(
                    vaug[:, tj, :, 0:64],
                    ps[:].rearrange("p (h d) -> p h d", h=HG))
                nc.gpsimd.dma_start(vaug[:, tj, :, 64:65], onesv.unsqueeze(-1))

            groups = ([(lambda ot=ot: qk_group(ot)) for ot in range(8)]
                      + [(lambda tt=tt: v_group(tt)) for tt in range(4)])
            return qt, groups

        def attention_chunk(j, qt, filler=()):
            filler = list(filler)
            nkt = 4 * j + 4
            opairs = []
            for p in range(4):          # head pair (2p, 2p+1)
                oo = [o_ps.tile([65, 512], F32, name="oo", tag="oo") for _ in range(2)]
                for kt in range(nkt):
                    dj = kt - 4 * j
                    q0 = max(dj, 0) * 128
                    nq = 512 - q0
                    # both heads' S^T into one 2-bank psum tile: bp 0/64
                    # row groups run concurrently on the PE when adjacent,
                    # and one wide exp covers both heads
                    sp = s_ps.tile([128, 1024], F32, name="sp", tag="sp")
                    for s in range(2):  # head 2p+s at base partition 64*s
                        bp = 64 * s
                        nc.tensor.matmul(
                            sp[:, 512 * s:512 * s + nq],
                            kT_sb[bp:bp + 64, p, kt * 128:(kt + 1) * 128],
                            qt[bp:bp + 64, p, q0:512],
                            start=True, stop=True,
                        )
                    e = e_pool.tile([128, 1024], F32R, name="e", tag="e")
                    sp3 = sp[:].rearrange("p (s q) -> p s q", s=2)
                    e3 = e[:].rearrange("p (s q) -> p s q", s=2)
                    nc.scalar.activation(e3[:, :, 0:nq], sp3[:, :, 0:nq], EXP,
                                         scale=0.125)
                    if dj >= 0:
                        nc.vector.tensor_mul(e3[:, :, 0:128], e3[:, :, 0:128],
                                             mask_sb[:].unsqueeze(1).broadcast_to((128, 2, 128)))
                    for s in range(2):
                        nc.tensor.matmul(
                            oo[s][:, q0:512], vaug[:, kt, 2 * p + s, :],
                            e[:, 512 * s:512 * s + nq],
                            start=(kt == 0), stop=(kt == nkt - 1),
                        )
                # normalize: O / rowsum -> opair [128, 512] (head a rows 0:64,
                # head b rows 64:128).  DVE ops support partition-shifted
                # outputs on HW, so no DMAs needed anywhere here.
                opair = o_pool.tile([128, 512], F32R)
                for s in range(2):
                    # denominator row: psum part 64 -> sbuf part 0 (DVE copy
                    # with shifted output), then fast reciprocal at part 0
                    # (recip_approx_fast only works from SBUF at partition 0)
                    r0 = r_pool.tile([1, 512], F32, tag="r0")
                    nc.vector.tensor_copy(r0[:], oo[s][64:65, :])
                    ri = r_pool.tile([1, 512], F32)
                    nc.vector.reciprocal_approx_fast(ri[:], r0[:])
                    rb = rb_pool.tile([128, 512], F32)
                    nc.gpsimd.partition_broadcast(rb[:], ri[:])
                    nc.vector.tensor_mul(opair[64 * s:64 * s + 64, :],
                                         oo[s][0:64, :], rb[0:64, :])
                opairs.append(opair)
                # weave filler (prev chunk's proj + next chunk's qkv) in
                take = (len(filler) + 3 - p) // (4 - p)
                for _ in range(take):
                    filler.pop(0)()

            def proj_group(ot):
                yp = acc_ps.tile([128, 512], F32, name="acc", tag="acc")
                for hh in range(4):
                    nc.tensor.matmul(
                        yp[:], wp_sb[:, hh, ot * 128:(ot + 1) * 128],
                        opairs[hh][:], start=(hh == 0), stop=(hh == 3),
                    )
                yo = y_sb.tile([128, 512], F32, tag="yout")
                nc.vector.tensor_copy(yo[:], yp[:])
                nc.sync.dma_start(
                    yT[ot * 128:(ot + 1) * 128, j * 512:(j + 1) * 512], yo[:])

            return [(lambda ot=ot: proj_group(ot)) for ot in range(8)]

        # software pipeline: chunk j+1's qkv groups are woven between
        # attention(j)'s head pairs as PE filler
        qt0, groups0 = qkv_chunk_groups(0)
        for g in groups0:
            g()
        qts = {0: qt0}
        carry = []
        for j in range(NCH):
            filler = list(carry)
            if j + 1 < NCH:
                qts[j + 1], qkv_filler = qkv_chunk_groups(j + 1)
                filler += qkv_filler
            carry = attention_chunk(j, qts.pop(j), filler)
        for g in carry:
            g()

    nc.compile()
    return nc


def _get_nc():
    if "nc" not in _CACHE:
        _CACHE["nc"] = _build()
    return _CACHE["nc"]


def kernel(x, w_qkv, w_proj, b_proj):
    from concourse.bass_utils import run_bass_kernel_spmd

    nc = _get_nc()
    x = np.asarray(x, np.float32)
    w_qkv = np.asarray(w_qkv, np.float32)
    w_proj = np.asarray(w_proj, np.float32)
    b_proj = np.asarray(b_proj, np.float32)

    kq = np.arange(128, dtype=np.float32)
    mask = (kq[:, None] <= kq[None, :]).astype(np.float32)   # [k, q]
    onesv = np.ones((128, HG), np.float32)

    in_maps = []
    for c in range(8):
        b, g = divmod(c, 2)
        wq = w_qkv[g * 512:(g + 1) * 512]
        wk = w_qkv[1024 + g * 512:1024 + (g + 1) * 512]
        wv = w_qkv[2048 + g * 512:2048 + (g + 1) * 512]
        in_maps.append({
            "xT": np.ascontiguousarray(x[b].T),
            "wqkT": np.ascontiguousarray(np.concatenate([wq, wk], 0).T),
            "wvT": np.ascontiguousarray(wv.T),
            "wpT": np.ascontiguousarray(w_proj[:, g * 512:(g + 1) * 512].T),
            "mask": mask,
            "onesv": onesv,
        })

    res = run_bass_kernel_spmd(nc, in_maps, core_ids=list(range(8)))
    out = np.empty((B, T, C), np.float32)
    for b in range(B):
        acc = res.results[2 * b]["yT"] + res.results[2 * b + 1]["yT"]
        out[b] = acc.T + b_proj
    return out


# revision 25
# speedup vs baseline: 1.0464x; 1.0464x over previous
"""Causal multi-head self-attention on 8 Trainium2 NeuronCores.

Problem: x[4, 2048, 1024], w_qkv[3072, 1024], w_proj[1024, 1024], b_proj[1024].
y = softmax_causal((xWq)(xWk)^T / 8) (xWv) -> concat heads -> @ w_proj^T + b.

Sharding: 8 cores = (batch b in 0..3) x (head-group g in 0..1), 8 heads per
core.  Each core computes the full attention for its (b, g) and a partial
output projection over its 512 head-dims; the host sums the two per-batch
partials, adds bias, and transposes back.

Everything on-chip lives in "transposed" space (feature dim on partitions):
the host passes x^T and pre-transposed weight slices, so the kernel needs no
on-chip transposes.  All matmuls use float32r (TF32-rate, ~1e-4 rel err).

Per-core dataflow, per t-chunk j of 512 tokens (j = 0..3):
  qkv:   qT/kT tiles  [64d x 2 heads, 512t] = W_qk^T-chunk.T @ xT-chunk
         V_aug tiles  [128t, 8 heads, 65]   = (xT-tile).T @ Wv^T  (+ ones col)
  attn (q-chunk = j, per head pair, per k-tile of 128):
         S^T [128k, Nq] = kT-slice.T @ qT-slice     (K=64, bp 0/64)
         E^T = exp(S^T/8)  (ScalarE, PSUM->SBUF, f32r out)
         tri-mask multiply on diagonal 128x128 sub-block (VectorE)
         O_aug [65, Nq] += V_aug.T @ E^T   (row 64 = softmax denominator)
  norm:  r = O_aug[64]; 1/r via DVE; broadcast over partitions via GpSimd;
         O_norm^T = O_aug[0:64] * (1/r)   -> staged [128, 512] per head pair
  proj:  yT [128o, 512q] += Wp^T-chunk.T @ O_norm^T-pair  -> DMA to DRAM
"""

import numpy as np

B, T, C = 4, 2048, 1024
HG, D = 8, 64           # heads per core, head dim
NCH = T // 512          # 4 t-chunks of 512
NKT = T // 128          # 16 k-tiles of 128
CCH = C // 128          # 8 contraction chunks for the qkv projection

_CACHE = {}


def _build():
    import concourse.bacc as bacc
    import concourse.mybir as mybir
    import concourse.tile as tile
    from contextlib import ExitStack

    F32 = mybir.dt.float32
    F32R = mybir.dt.float32r
    EXP = mybir.ActivationFunctionType.Exp

    nc = bacc.Bacc("TRN2", target_bir_lowering=False, debug=False)

    xT = nc.dram_tensor("xT", [C, T], F32R, kind="ExternalInput").ap()
    wqkT = nc.dram_tensor("wqkT", [C, 1024], F32R, kind="ExternalInput").ap()
    wvT = nc.dram_tensor("wvT", [C, 512], F32R, kind="ExternalInput").ap()
    wpT = nc.dram_tensor("wpT", [512, 1024], F32R, kind="ExternalInput").ap()
    mask = nc.dram_tensor("mask", [128, 128], F32, kind="ExternalInput").ap()
    onesv = nc.dram_tensor("onesv", [128, HG], F32R, kind="ExternalInput").ap()
    yT = nc.dram_tensor("yT", [C, T], F32, kind="ExternalOutput").ap()

    with tile.TileContext(nc) as tc, ExitStack() as ctx:
        consts = ctx.enter_context(tc.tile_pool(name="consts", bufs=1))
        xt_pool = ctx.enter_context(tc.tile_pool(name="xt", bufs=1))
        qt_pool = ctx.enter_context(tc.tile_pool(name="qt", bufs=2))
        kv_pool = ctx.enter_context(tc.tile_pool(name="kv", bufs=1))
        e_pool = ctx.enter_context(tc.tile_pool(name="e", bufs=3))
        o_pool = ctx.enter_context(tc.tile_pool(name="o", bufs=8))
        r_pool = ctx.enter_context(tc.tile_pool(name="r", bufs=1))
        rb_pool = ctx.enter_context(tc.tile_pool(name="rb", bufs=1))
        y_sb = ctx.enter_context(tc.tile_pool(name="y_sb", bufs=2))

        # 8 psum banks: s-pair tiles 2x[128,1024] (4) + shared qkv/proj
        # accumulators (2) + O-pair accumulators (2)
        acc_ps = ctx.enter_context(tc.tile_pool(name="acc_ps", bufs=2, space="PSUM"))
        s_ps = ctx.enter_context(tc.tile_pool(name="s_ps", bufs=2, space="PSUM"))
        o_ps = ctx.enter_context(tc.tile_pool(name="o_ps", bufs=2, space="PSUM"))

        # ---- resident constants / weights ----
        # split weight loads per output-column group so the first qkv matmul
        # group only waits for its own 512KB slice
        wqk_sb = consts.tile([128, CCH, 1024], F32R)
        wqkT_r = wqkT.rearrange("(cc p) o -> p cc o", p=128)
        wv_sb = consts.tile([128, CCH, 512], F32R)
        wvT_r = wvT.rearrange("(cc p) o -> p cc o", p=128)
        # weights go on idle engines' DMA queues so the sync queue leads
        # with the x chunks the first matmuls need
        for ot in range(8):
            nc.gpsimd.dma_start(wqk_sb[:, :, ot * 128:(ot + 1) * 128],
                                wqkT_r[:, :, ot * 128:(ot + 1) * 128])
        for cc in range(CCH):
            nc.gpsimd.dma_start(wv_sb[:, cc, :], wvT_r[:, cc, :])
        wp_sb = consts.tile([128, 4, 1024], F32R)
        nc.gpsimd.dma_start(wp_sb[:], wpT.rearrange("(hh p) o -> p hh o", p=128))
        mask_sb = consts.tile([128, 128], F32)
        nc.gpsimd.dma_start(mask_sb[:], mask)

        # K (all chunks) and V_aug (all t-tiles) stay resident
        kT_sb = consts.tile([128, 4, T], F32R)            # [2 heads*64d, pair, t]
        vaug = consts.tile([128, NKT, HG, 65], F32R)      # [t, k-tile, head, d+1]

        xT_r = xT.rearrange("(cc p) t -> p cc t", p=128)

        def qkv_chunk_groups(j):
            """Returns (qt_tile, [group closures]) for t-chunk j.  Each
            closure emits one 8-matmul psum accumulation group; calling them
            interleaved with attention keeps the PE dense."""
            xt = xt_pool.tile([128, CCH, 512], F32R, name="xt", tag="xt")
            for cc in range(CCH):
                nc.sync.dma_start(xt[:, cc, :],
                                  xT_r[:, cc, j * 512:(j + 1) * 512])
            qt = qt_pool.tile([128, 4, 512], F32R, name="qt", tag="qt")

            def qk_group(ot):
                ps = acc_ps.tile([128, 512], F32, name="acc", tag="acc")
                for cc in range(CCH):
                    nc.tensor.matmul(
                        ps[:], wqk_sb[:, cc, ot * 128:(ot + 1) * 128],
                        xt[:, cc, :], start=(cc == 0), stop=(cc == CCH - 1),
                    )
                if ot < 4:
                    nc.vector.tensor_copy(qt[:, ot, :], ps[:])
                else:
                    nc.vector.tensor_copy(
                        kT_sb[:, ot - 4, j * 512:(j + 1) * 512], ps[:])

            def v_group(tt):
                tj = 4 * j + tt
                ps = acc_ps.tile([128, 512], F32, name="acc", tag="acc")
                for cc in range(CCH):
                    nc.tensor.matmul(
                        ps[:], xt[:, cc, tt * 128:(tt + 1) * 128],
                        wv_sb[:, cc, :], start=(cc == 0), stop=(cc == CCH - 1),
                    )
                nc.vector.tensor_copy(
                    vaug[:, tj, :, 0:64],
                    ps[:].rearrange("p (h d) -> p h d", h=HG))
                nc.gpsimd.dma_start(vaug[:, tj, :, 64:65], onesv.unsqueeze(-1))

            groups = ([(lambda ot=ot: qk_group(ot)) for ot in range(8)]
                      + [(lambda tt=tt: v_group(tt)) for tt in range(4)])
            return qt, groups

        def attention_chunk(j, qt, filler=()):
            filler = list(filler)
            nkt = 4 * j + 4
            opairs = []
            for p in range(4):          # head pair (2p, 2p+1)
                oo = [o_ps.tile([65, 512], F32, name="oo", tag="oo") for _ in range(2)]
                for kt in range(nkt):
                    dj = kt - 4 * j
                    q0 = max(dj, 0) * 128
                    nq = 512 - q0
                    # both heads' S^T into one 2-bank psum tile: bp 0/64
                    # row groups run concurrently on the PE when adjacent,
                    # and one wide exp covers both heads
                    sp = s_ps.tile([128, 1024], F32, name="sp", tag="sp")
                    for s in range(2):  # head 2p+s at base partition 64*s
                        bp = 64 * s
                        nc.tensor.matmul(
                            sp[:, 512 * s:512 * s + nq],
                            kT_sb[bp:bp + 64, p, kt * 128:(kt + 1) * 128],
                            qt[bp:bp + 64, p, q0:512],
                            start=True, stop=True,
                        )
                    e = e_pool.tile([128, 1024], F32R, name="e", tag="e")
                    sp3 = sp[:].rearrange("p (s q) -> p s q", s=2)
                    e3 = e[:].rearrange("p (s q) -> p s q", s=2)
                    nc.scalar.activation(e3[:, :, 0:nq], sp3[:, :, 0:nq], EXP,
                                         scale=0.125)
                    if dj >= 0:
                        nc.vector.tensor_mul(e3[:, :, 0:128], e3[:, :, 0:128],
                                             mask_sb[:].unsqueeze(1).broadcast_to((128, 2, 128)))
                    for s in range(2):
                        nc.tensor.matmul(
                            oo[s][:, q0:512], vaug[:, kt, 2 * p + s, :],
                            e[:, 512 * s:512 * s + nq],
                            start=(kt == 0), stop=(kt == nkt - 1),
                        )
                # normalize: O / rowsum -> opair [128, 512] (head a rows 0:64,
                # head b rows 64:128).  DVE ops support partition-shifted
                # outputs on HW, so no DMAs needed anywhere here.
                opair = o_pool.tile([128, 512], F32R)
                for s in range(2):
                    # denominator row: psum part 64 -> sbuf part 0 (DVE copy
                    # with shifted output), then fast reciprocal at part 0
                    # (recip_approx_fast only works from SBUF at partition 0)
                    r0 = r_pool.tile([1, 512], F32, tag="r0")
                    nc.vector.tensor_copy(r0[:], oo[s][64:65, :])
                    ri = r_pool.tile([1, 512], F32)
                    nc.vector.reciprocal_approx_fast(ri[:], r0[:])
                    rb = rb_pool.tile([128, 512], F32)
                    nc.gpsimd.partition_broadcast(rb[:], ri[:])
                    nc.vector.tensor_mul(opair[64 * s:64 * s + 64, :],
                                         oo[s][0:64, :], rb[0:64, :])
                opairs.append(opair)
                # weave filler (prev chunk's proj + next chunk's qkv) in
                take = (len(filler) + 3 - p) // (4 - p)
                for _ in range(take):
                    filler.pop(0)()

            def proj_group(ot):
                yp = acc_ps.tile([128, 512], F32, name="acc", tag="acc")
                for hh in range(4):
                    nc.tensor.matmul(
                        yp[:], wp_sb[:, hh, ot * 128:(ot + 1) * 128],
                        opairs[hh][:], start=(hh == 0), stop=(hh == 3),
                    )
                yo = y_sb.tile([128, 512], F32, tag="yout")
                nc.vector.tensor_copy(yo[:], yp[:])
                nc.sync.dma_start(
                    yT[ot * 128:(ot + 1) * 128, j * 512:(j + 1) * 512], yo[:])

            return [(lambda ot=ot: proj_group(ot)) for ot in range(8)]

        # software pipeline: chunk j+1's qkv groups are woven between
        # attention(j)'s head pairs as PE filler
        qt0, groups0 = qkv_chunk_groups(0)
        for g in groups0:
            g()
        qts = {0: qt0}
        carry = []
        for j in range(NCH):
            filler = list(carry)
            if j + 1 < NCH:
                qts[j + 1], qkv_filler = qkv_chunk_groups(j + 1)
                filler += qkv_filler
            carry = attention_chunk(j, qts.pop(j), filler)
        for g in carry:
            g()

    nc.compile()
    return nc


def _get_nc():
    if "nc" not in _CACHE:
        _CACHE["nc"] = _build()
    return _CACHE["nc"]


def kernel(x, w_qkv, w_proj, b_proj):
    from concourse.bass_utils import run_bass_kernel_spmd

    nc = _get_nc()
    x = np.asarray(x, np.float32)
    w_qkv = np.asarray(w_qkv, np.float32)
    w_proj = np.asarray(w_proj, np.float32)
    b_proj = np.asarray(b_proj, np.float32)

    kq = np.arange(128, dtype=np.float32)
    mask = (kq[:, None] <= kq[None, :]).astype(np.float32)   # [k, q]
    onesv = np.ones((128, HG), np.float32)

    in_maps = []
    for c in range(8):
        b, g = divmod(c, 2)
        wq = w_qkv[g * 512:(g + 1) * 512]
        wk = w_qkv[1024 + g * 512:1024 + (g + 1) * 512]
        wv = w_qkv[2048 + g * 512:2048 + (g + 1) * 512]
        in_maps.append({
            "xT": np.ascontiguousarray(x[b].T),
            "wqkT": np.ascontiguousarray(np.concatenate([wq, wk], 0).T),
            "wvT": np.ascontiguousarray(wv.T),
            "wpT": np.ascontiguousarray(w_proj[:, g * 512:(g + 1) * 512].T),
            "mask": mask,
            "onesv": onesv,
        })

    res = run_bass_kernel_spmd(nc, in_maps, core_ids=list(range(8)))
    out = np.empty((B, T, C), np.float32)
    for b in range(B):
        acc = res.results[2 * b]["yT"] + res.results[2 * b + 1]["yT"]
        out[b] = acc.T + b_proj
    return out
